# revision 7
# baseline (speedup 1.0000x reference)
"""Multi-head causal self-attention (B=2, T=2048, C=1024, H=16, D=64) on 8
Trainium2 NeuronCores.

Sharding: data-parallel over batch (2) x tensor-parallel over heads (4 groups
of 4 heads) = 8 shards, no cross-core communication. Each core computes, for
its (batch b, head-group g):
    qkvT = w_slice.T @ x[b].T  (+bias)   -> qT,kT [256,2048], v [2048,256]
    per head: scoresT = kT.T' qT ... softmax (transposed layout, causal)
    attT (unnormalized) + denominators via a ones-column in the PV matmul
    partial output = attT_norm.T @ w_proj_rows + 0.25*b_proj  -> [2048,1024]
Host sums the 4 partial outputs per batch.

All matmuls run in float32r (fp32 truncated to ~fp22 in the PE), which is
full-rate on TRN2 for moving free dim >= 256.
"""

import numpy as np

import concourse.bass as bass
import concourse.mybir as mybir
import concourse.tile as tile
from concourse import bacc
from concourse.bass_utils import run_bass_kernel_spmd

f32 = mybir.dt.float32
f32r = mybir.dt.float32r
AF = mybir.ActivationFunctionType
ALU = mybir.AluOpType

B, T, C, H, D = 2, 2048, 1024, 16, 64
HPC = 4          # heads per core
NCORES = 8
TQ = 512         # q tile (matmul moving free dim)
TK = 128         # k tile (psum partition dim)
NTQ = T // TQ    # 4
NKC = C // 128   # 8 contraction chunks for the qkv projection
SCALE = 1.0 / 8.0  # 1/sqrt(D)

_CACHE = {}

U32_ONE = 1065353216  # np.float32(1.0).view(np.uint32)


def memset_bits(eng, ap, bits):
    """memset an fp32r AP via its uint32 bit view (ISA has no fp32r memset)."""
    eng.memset(ap.bitcast(mybir.dt.uint32), bits)



def r(ap):
    return ap.bitcast(f32r)


def build_nc(debug_taps=False):
    nc = bacc.Bacc("TRN2", target_bir_lowering=False, debug=False)

    xt_d = nc.dram_tensor("xt", [C, T], f32r, kind="ExternalInput")
    wqkv_d = nc.dram_tensor("wqkv", [C, 768], f32r, kind="ExternalInput")
    bqk_d = nc.dram_tensor("bqk", [128, 4], f32, kind="ExternalInput")
    bv_d = nc.dram_tensor("bv", [1, 256], f32r, kind="ExternalInput")
    wproj_d = nc.dram_tensor("wproj", [256, C], f32r, kind="ExternalInput")
    bprojq_d = nc.dram_tensor("bprojq", [1, C], f32r, kind="ExternalInput")
    out_d = nc.dram_tensor("out", [T, C], f32, kind="ExternalOutput")
    if debug_taps:
        dbg_qkT = nc.dram_tensor("dbg_qkT", [128, 4, T], f32, kind="ExternalOutput")
        dbg_v = nc.dram_tensor("dbg_v", [128, T // 128, HPC, 128], f32, kind="ExternalOutput")
        dbg_attT = nc.dram_tensor("dbg_attT", [128, 2, T], f32, kind="ExternalOutput")
        dbg_pt = nc.dram_tensor("dbg_pt", [4, 128, TQ], f32, kind="ExternalOutput")
        dbg_acc = nc.dram_tensor("dbg_acc", [65, TQ], f32, kind="ExternalOutput")
        dbg_bcs = nc.dram_tensor("dbg_bcs", [64, TQ], f32, kind="ExternalOutput")
        dbg_rec = nc.dram_tensor("dbg_rec", [2, TQ], f32, kind="ExternalOutput")

    with tile.TileContext(nc) as tc:
        with (
            tc.tile_pool(name="const", bufs=1) as const,
            tc.tile_pool(name="xts", bufs=2) as xts_pool,
            tc.tile_pool(name="pt", bufs=6) as pt_pool,
            tc.tile_pool(name="bcs", bufs=4) as bcs_pool,
            tc.tile_pool(name="rec", bufs=2) as rec_pool,
            tc.tile_pool(name="ot", bufs=4) as ot_pool,
            tc.tile_pool(name="ps_a", bufs=2, space="PSUM") as ps_a,
            tc.tile_pool(name="ps_s", bufs=4, space="PSUM") as ps_s,
            tc.tile_pool(name="ps_acc", bufs=2, space="PSUM") as ps_acc,
        ):
            # ---- resident tensors -------------------------------------
            wqkv_sb = const.tile([128, NKC, 768], f32r, tag="wqkv")
            nc.sync.dma_start(wqkv_sb[:], wqkv_d.rearrange("(o p) n -> p o n", p=128))
            bqk_sb = const.tile([128, 4], f32, tag="bqk")
            nc.sync.dma_start(bqk_sb[:], bqk_d[:, :])
            bv_sb = const.tile([1, 256], f32r, tag="bv")
            nc.sync.dma_start(bv_sb[:], bv_d[:, :])
            wproj_sb = const.tile([128, 2, C], f32r, tag="wproj")
            nc.sync.dma_start(wproj_sb[:], wproj_d.rearrange("(o p) n -> p o n", p=128))
            bprojq_sb = const.tile([1, C], f32r, tag="bprojq")
            nc.sync.dma_start(bprojq_sb[:], bprojq_d[:, :])

            ones_sb = const.tile([128, 128], f32r, tag="ones")
            memset_bits(nc.vector, ones_sb[:], U32_ONE)
            # onespad: col 0-63 = 0, 64-127 = 1 (for odd-head recip broadcast)
            onespad_sb = const.tile([1, 128], f32r, tag="onespad")
            memset_bits(nc.vector, onespad_sb[:, 0:64], 0)
            memset_bits(nc.vector, onespad_sb[:, 64:128], U32_ONE)

            # qkT: chunk 0,1 = qT (heads 01 / 23), chunk 2,3 = kT
            qkT_sb = const.tile([128, 4, T], f32r, tag="qkT")
            # v natural layout + PV lhsT padding:
            #   even head h: cols [V(64) | ones(1) | unused(63)]
            #   odd head h:  cols [ones(1) | zeros(63) | V(64)]
            v_sb = const.tile([128, T // 128, HPC, 128], f32r, tag="v")
            memset_bits(nc.vector, v_sb[:], 0)
            for h in range(HPC):
                col = 64 if h % 2 == 0 else 0
                memset_bits(nc.vector, v_sb[:, :, h, col : col + 1], U32_ONE)
            # attT: chunk c partitions 0-63 = head 2c, 64-127 = head 2c+1
            attT_sb = const.tile([128, 2, T], f32r, tag="attT")

            # ---- phase 1: qkv projection ------------------------------
            for tq in range(NTQ):
                tqs = slice(TQ * tq, TQ * (tq + 1))
                xts = xts_pool.tile([128, NKC, TQ], f32r, tag="xts")
                nc.sync.dma_start(
                    xts[:], xt_d.rearrange("(o p) t -> p o t", p=128)[:, :, tqs]
                )
                # q,k transposed: psum = wqkv_chunk.T @ xT
                for cp in range(4):
                    ps = ps_a.tile([128, TQ], f32, tag="ps_a")
                    for kc in range(NKC):
                        nc.tensor.matmul(
                            ps[:],
                            lhsT=wqkv_sb[:, kc, 128 * cp : 128 * (cp + 1)],
                            rhs=xts[:, kc, :],
                            start=(kc == 0),
                            stop=(kc == NKC - 1),
                        )
                    # copyback + per-partition bias on DVE
                    nc.vector.tensor_scalar_add(
                        qkT_sb[:, cp, tqs], ps[:], bqk_sb[:, cp : cp + 1]
                    )
                # v natural: psum = xT_chunk.T @ wv (+ bias via K=1 matmul)
                for tt in range(4 * tq, 4 * tq + 4):
                    psv = ps_a.tile([128, TQ], f32, tag="ps_a")
                    toff = 128 * tt - TQ * tq
                    for kc in range(NKC):
                        nc.tensor.matmul(
                            psv[:, 0:256],
                            lhsT=xts[:, kc, toff : toff + 128],
                            rhs=wqkv_sb[:, kc, 512:768],
                            start=(kc == 0),
                            stop=False,
                        )
                    nc.tensor.matmul(
                        psv[:, 0:256],
                        lhsT=ones_sb[0:1, 0:128],
                        rhs=bv_sb[0:1, :],
                        start=False,
                        stop=True,
                    )
                    for h in range(HPC):
                        col = 0 if h % 2 == 0 else 64
                        nc.vector.tensor_copy(
                            v_sb[:, tt, h, col : col + 64],
                            psv[:, 64 * h : 64 * h + 64],
                        )

            # ---- phase 2: attention -----------------------------------
            for pair in range(2):
                hA, hB = 2 * pair, 2 * pair + 1
                for tq in range(NTQ):
                    tqs = slice(TQ * tq, TQ * (tq + 1))
                    accA = ps_acc.tile([128, TQ], f32, tag="acc")
                    accB = ps_acc.tile([128, TQ], f32, tag="acc")
                    ntk = 4 * tq + 4
                    for tk in range(ntk):
                        d = tk - 4 * tq  # >= 0 on the diagonal block
                        q0 = 128 * d if d >= 0 else 0
                        w = TQ - q0
                        ks = slice(128 * tk, 128 * (tk + 1))
                        qs = slice(TQ * tq + q0, TQ * (tq + 1))
                        scA = ps_s.tile([128, TQ], f32, tag="sc")
                        scB = ps_s.tile([128, TQ], f32, tag="sc")
                        # two K=64 matmuls packed on row halves of the PE
                        nc.tensor.matmul(
                            scA[:, 0:w],
                            lhsT=qkT_sb[0:64, 2 + pair, ks],
                            rhs=qkT_sb[0:64, pair, qs],
                        )
                        nc.tensor.matmul(
                            scB[:, 0:w],
                            lhsT=qkT_sb[64:128, 2 + pair, ks],
                            rhs=qkT_sb[64:128, pair, qs],
                        )
                        ptA = pt_pool.tile([128, TQ], f32r, tag="pt")
                        ptB = pt_pool.tile([128, TQ], f32r, tag="pt")
                        nc.scalar.activation(ptA[:, 0:w], scA[:, 0:w], AF.Exp, scale=SCALE)
                        nc.scalar.activation(ptB[:, 0:w], scB[:, 0:w], AF.Exp, scale=SCALE)
                        if d >= 0:
                            # zero strictly-above-diagonal in the 128x128 block
                            # keep where (j - p) >= 0
                            for pt_ in (ptA, ptB):
                                nc.gpsimd.affine_select(
                                    pt_[:, 0:128],
                                    pt_[:, 0:128],
                                    pattern=[[1, 128]],
                                    compare_op=ALU.is_ge,
                                    fill=0.0,
                                    base=0,
                                    channel_multiplier=-1,
                                )
                        if debug_taps and pair == 0 and tq == 0:
                            dtp = pt_pool.tile([128, TQ], f32, tag="dbgcp")
                            nc.vector.tensor_copy(dtp[:, 0:w], ptA[:, 0:w])
                            if q0 > 0:
                                nc.vector.memset(dtp[:, 0:q0], 0.0)
                            nc.sync.dma_start(dbg_pt[tk, :, :], dtp[:])
                        st = (tk == 0)
                        sp = (tk == ntk - 1)
                        # even head: out rows 0-63 att, row 64 denom
                        nc.tensor.matmul(
                            accA[0:65, q0:TQ],
                            lhsT=v_sb[:, tk, hA, 0:65],
                            rhs=ptA[:, 0:w],
                            start=st,
                            stop=sp,
                            skip_group_check=True,
                        )
                        # odd head: out row 0 denom, rows 64-127 att
                        nc.tensor.matmul(
                            accB[:, q0:TQ],
                            lhsT=v_sb[:, tk, hB, :],
                            rhs=ptB[:, 0:w],
                            start=st,
                            stop=sp,
                            skip_group_check=True,
                        )
                    if debug_taps and pair == 0 and tq == 0:
                        dta = pt_pool.tile([128, TQ], f32, tag="dbgcp")
                        nc.vector.tensor_copy(dta[0:65, :], accA[0:65, :])
                        nc.sync.dma_start(dbg_acc[:, :], dta[0:65, :])
                    # normalize head A (denominator at partition 64).
                    # reciprocal_approx_fast is broken at base_partition != 0 on
                    # HW, so broadcast the raw denominator down to rows 0-63
                    # first and take the reciprocal there.
                    denA = rec_pool.tile([128, TQ], f32r, tag="rec")
                    nc.vector.tensor_copy(denA[64:65, :], accA[64:65, :])
                    bcA = ps_a.tile([128, TQ], f32, tag="ps_a")
                    nc.tensor.matmul(
                        bcA[0:64, :],
                        lhsT=ones_sb[64:65, 0:64],
                        rhs=denA[64:65, :],
                    )
                    bcsA = bcs_pool.tile([128, TQ], f32, tag="bcs")
                    nc.scalar.activation(bcsA[0:64, :], bcA[0:64, :], AF.Copy)
                    nc.vector.reciprocal_approx_fast(
                        out=bcsA[0:64, :], in_=bcsA[0:64, :]
                    )
                    if debug_taps and pair == 0 and tq == 0:
                        nc.sync.dma_start(dbg_rec[0:1, :], bcsA[0:1, :])
                        nc.sync.dma_start(dbg_rec[1:2, :], denA[64:65, :].bitcast(f32))
                    nc.vector.tensor_mul(
                        attT_sb[0:64, pair, tqs], accA[0:64, :], bcsA[0:64, :]
                    )
                    if debug_taps and pair == 0 and tq == 0:
                        nc.sync.dma_start(dbg_bcs[:, :], bcsA[0:64, :])
                    # normalize head B (denominator at partition 0)
                    recB = rec_pool.tile([128, TQ], f32, tag="rec")
                    nc.vector.reciprocal_approx_fast(
                        out=recB[0:1, :], in_=accB[0:1, :]
                    )
                    recBr = rec_pool.tile([128, TQ], f32r, tag="recr")
                    nc.vector.tensor_copy(recBr[0:1, :], recB[0:1, :])
                    bcB = ps_a.tile([128, TQ], f32, tag="ps_a")
                    nc.tensor.matmul(
                        bcB[:, :],
                        lhsT=onespad_sb[0:1, :],
                        rhs=recBr[0:1, :],
                    )
                    bcsB = bcs_pool.tile([128, TQ], f32, tag="bcs")
                    nc.scalar.activation(bcsB[64:128, :], bcB[64:128, :], AF.Copy)
                    nc.vector.tensor_mul(
                        attT_sb[64:128, pair, tqs], accB[64:128, :], bcsB[64:128, :]
                    )

            if debug_taps:
                nc.sync.dma_start(dbg_qkT[:, :, :], qkT_sb[:].bitcast(f32))
                nc.sync.dma_start(dbg_v[:, :, :, :], v_sb[:].bitcast(f32))
                nc.sync.dma_start(dbg_attT[:, :, :], attT_sb[:].bitcast(f32))

            # ---- phase 3: output projection ---------------------------
            for tt in range(T // 128):
                ts_ = slice(128 * tt, 128 * (tt + 1))
                for nt in range(2):
                    ns = slice(512 * nt, 512 * (nt + 1))
                    pso = ps_a.tile([128, TQ], f32, tag="ps_a")
                    for hc in range(2):
                        nc.tensor.matmul(
                            pso[:],
                            lhsT=attT_sb[:, hc, ts_],
                            rhs=wproj_sb[:, hc, ns],
                            start=(hc == 0),
                            stop=False,
                        )
                    nc.tensor.matmul(
                        pso[:],
                        lhsT=ones_sb[0:1, 0:128],
                        rhs=bprojq_sb[0:1, ns],
                        start=False,
                        stop=True,
                    )
                    ot = ot_pool.tile([128, TQ], f32, tag="ot")
                    if nt == 0:
                        nc.vector.tensor_copy(ot[:], pso[:])
                    else:
                        nc.scalar.activation(ot[:], pso[:], AF.Copy)
                    nc.sync.dma_start(out_d[ts_, ns], ot[:])

    nc.compile()
    return nc


def _shard_inputs(x, w_qkv, b_qkv, w_proj, b_proj):
    """Full inputs -> per-core input maps. Core c = (batch b=c//4, group g=c%4)."""
    in_maps = []
    xts = [np.ascontiguousarray(x[b].T) for b in range(B)]
    bprojq = np.ascontiguousarray((0.25 * b_proj).reshape(1, C).astype(np.float32))
    for core in range(NCORES):
        b, g = divmod(core, 4)
        qs = slice(256 * g, 256 * (g + 1))
        ks = slice(C + 256 * g, C + 256 * (g + 1))
        vs = slice(2 * C + 256 * g, 2 * C + 256 * (g + 1))
        wqkv = np.ascontiguousarray(
            np.concatenate([w_qkv[:, qs], w_qkv[:, ks], w_qkv[:, vs]], axis=1)
        )
        bqk = np.ascontiguousarray(
            np.stack(
                [
                    b_qkv[qs][0:128],
                    b_qkv[qs][128:256],
                    b_qkv[ks][0:128],
                    b_qkv[ks][128:256],
                ],
                axis=1,
            )
        )
        bv = np.ascontiguousarray(b_qkv[vs].reshape(1, 256))
        wproj = np.ascontiguousarray(w_proj[256 * g : 256 * (g + 1), :])
        in_maps.append(
            {
                "xt": xts[b],
                "wqkv": wqkv.astype(np.float32),
                "bqk": bqk.astype(np.float32),
                "bv": bv.astype(np.float32),
                "wproj": wproj.astype(np.float32),
                "bprojq": bprojq,
            }
        )
    return in_maps


def kernel(x, w_qkv, b_qkv, w_proj, b_proj):
    x = np.asarray(x, dtype=np.float32)
    w_qkv = np.asarray(w_qkv, dtype=np.float32)
    b_qkv = np.asarray(b_qkv, dtype=np.float32)
    w_proj = np.asarray(w_proj, dtype=np.float32)
    b_proj = np.asarray(b_proj, dtype=np.float32)

    if "nc" not in _CACHE:
        _CACHE["nc"] = build_nc()
    nc = _CACHE["nc"]

    in_maps = _shard_inputs(x, w_qkv, b_qkv, w_proj, b_proj)
    res = run_bass_kernel_spmd(nc, in_maps, list(range(NCORES)))
    out = np.empty((B, T, C), dtype=np.float32)
    for b in range(B):
        acc = res.results[4 * b]["out"].astype(np.float32)
        for g in range(1, 4):
            acc = acc + res.results[4 * b + g]["out"]
        out[b] = acc
    return out


# revision 8
# speedup vs baseline: 1.0201x; 1.0201x over previous
"""Multi-head causal self-attention (B=2, T=2048, C=1024, H=16, D=64) on 8
Trainium2 NeuronCores.

Sharding: data-parallel over batch (2) x tensor-parallel over heads (4 groups
of 4 heads) = 8 shards, no cross-core communication. Each core computes, for
its (batch b, head-group g):
    qkvT = w_slice.T @ x[b].T  (+bias)   -> qT,kT [256,2048], v [2048,256]
    per head: scoresT = kT' q ... softmax in transposed layout (causal)
    attT (unnormalized) + denominators via a ones-column in the PV matmul
    partial output = attT_norm.T @ w_proj_rows + 0.25*b_proj  -> [2048,1024]
Host sums the 4 partial outputs per batch.

Projection / score matmuls run in float32r (fp32 truncated to ~fp22 in the
PE - full rate for moving free dim >= 256). The probability*value matmul runs
in bf16 (P quantization ~0.4%) which keeps the exp on the scalar engine at 2x
and the PV matmul off the fp32r small-N penalty.

The per-tq blocks interleave qkv projection, attention, and output projection
so every engine has work throughout instead of three serial phases.
"""

import numpy as np

import concourse.bass as bass
import concourse.mybir as mybir
import concourse.tile as tile
from concourse import bacc
from concourse.bass_utils import run_bass_kernel_spmd

f32 = mybir.dt.float32
f32r = mybir.dt.float32r
bf16 = mybir.dt.bfloat16
AF = mybir.ActivationFunctionType
ALU = mybir.AluOpType

B, T, C, H, D = 2, 2048, 1024, 16, 64
HPC = 4          # heads per core
NCORES = 8
TQ = 512         # q tile (matmul moving free dim)
NTQ = T // TQ    # 4
NKC = C // 128   # 8 contraction chunks for the qkv projection
SCALE = 1.0 / 8.0  # 1/sqrt(D)

USE_BF16_PV = True
PV_DT = bf16 if USE_BF16_PV else f32r

_CACHE = {}

U32_ONE = 1065353216  # np.float32(1.0).view(np.uint32)


def memset_bits(eng, ap, bits):
    """memset an fp32r AP via its uint32 bit view (ISA has no fp32r memset)."""
    eng.memset(ap.bitcast(mybir.dt.uint32), bits)


def pv_memset(eng, ap, val):
    if USE_BF16_PV:
        eng.memset(ap, val)
    else:
        memset_bits(eng, ap, U32_ONE if val == 1.0 else 0)


def build_nc(debug_taps=False):
    nc = bacc.Bacc("TRN2", target_bir_lowering=False, debug=False)

    xt_d = nc.dram_tensor("xt", [C, T], f32r, kind="ExternalInput")
    wqkv_d = nc.dram_tensor("wqkv", [C, 768], f32r, kind="ExternalInput")
    bqk_d = nc.dram_tensor("bqk", [128, 4], f32, kind="ExternalInput")
    bv_d = nc.dram_tensor("bv", [1, 256], f32r, kind="ExternalInput")
    wproj_d = nc.dram_tensor("wproj", [256, C], f32r, kind="ExternalInput")
    bprojq_d = nc.dram_tensor("bprojq", [1, C], f32r, kind="ExternalInput")
    out_d = nc.dram_tensor("out", [T, C], f32, kind="ExternalOutput")
    if debug_taps:
        dbg_qkT = nc.dram_tensor("dbg_qkT", [128, 4, T], f32, kind="ExternalOutput")
        dbg_attT = nc.dram_tensor("dbg_attT", [128, 2, T], f32, kind="ExternalOutput")
        dbg_pt = nc.dram_tensor("dbg_pt", [4, 128, TQ], f32, kind="ExternalOutput")

    with tile.TileContext(nc) as tc:
        with (
            tc.tile_pool(name="const", bufs=1) as const,
            tc.tile_pool(name="xts", bufs=2) as xts_pool,
            tc.tile_pool(name="pt", bufs=6) as pt_pool,
            tc.tile_pool(name="bcs", bufs=4) as bcs_pool,
            tc.tile_pool(name="rec", bufs=4) as rec_pool,
            tc.tile_pool(name="ot", bufs=4) as ot_pool,
            tc.tile_pool(name="ps_a", bufs=2, space="PSUM") as ps_a,
            tc.tile_pool(name="ps_s", bufs=3, space="PSUM") as ps_s,
            tc.tile_pool(name="ps_acc", bufs=3, space="PSUM") as ps_acc,
        ):
            # ---- resident tensors; DMAs chunked so compute starts early ----
            wqkv_sb = const.tile([128, NKC, 768], f32r, tag="wqkv")
            wqkv_r = wqkv_d.rearrange("(o p) n -> p o n", p=128)
            for kc in range(NKC):
                nc.sync.dma_start(wqkv_sb[:, kc, :], wqkv_r[:, kc, :])
            bqk_sb = const.tile([128, 4], f32, tag="bqk")
            nc.sync.dma_start(bqk_sb[:], bqk_d[:, :])
            bv_sb = const.tile([1, 256], f32r, tag="bv")
            nc.sync.dma_start(bv_sb[:], bv_d[:, :])
            wproj_sb = const.tile([128, 2, C], f32r, tag="wproj")
            nc.sync.dma_start(wproj_sb[:], wproj_d.rearrange("(o p) n -> p o n", p=128))
            bprojq_sb = const.tile([1, C], f32r, tag="bprojq")
            nc.sync.dma_start(bprojq_sb[:], bprojq_d[:, :])

            ones_sb = const.tile([128, 128], f32r, tag="ones")
            memset_bits(nc.vector, ones_sb[:], U32_ONE)
            # onespad: col 0-63 = 0, 64-127 = 1 (for odd-head recip broadcast)
            onespad_sb = const.tile([1, 128], f32r, tag="onespad")
            memset_bits(nc.vector, onespad_sb[:, 0:64], 0)
            memset_bits(nc.vector, onespad_sb[:, 64:128], U32_ONE)

            # qkT: chunk 0,1 = qT (heads 01 / 23), chunk 2,3 = kT
            qkT_sb = const.tile([128, 4, T], f32r, tag="qkT")
            # v (PV lhsT layout), per head h:
            #   even h: cols [V(64) | ones(1)]            (rest unused)
            #   odd h:  cols [ones(1) | zeros(63) | V(64)]
            v_sb = const.tile([128, T // 128, HPC, 128], PV_DT, tag="v")
            for h in range(HPC):
                if h % 2 == 0:
                    pv_memset(nc.vector, v_sb[:, :, h, 64:65], 1.0)
                else:
                    pv_memset(nc.vector, v_sb[:, :, h, 1:64], 0.0)
                    pv_memset(nc.vector, v_sb[:, :, h, 0:1], 1.0)
            # attT: chunk c partitions 0-63 = head 2c, 64-127 = head 2c+1
            attT_sb = const.tile([128, 2, T], f32r, tag="attT")

            xt_r = xt_d.rearrange("(o p) t -> p o t", p=128)

            for tq in range(NTQ):
                tqs = slice(TQ * tq, TQ * (tq + 1))

                # ---- qkv projection for this t-slice ----------------------
                xts = xts_pool.tile([128, NKC, TQ], f32r, tag="xts")
                for kc in range(NKC):
                    nc.sync.dma_start(xts[:, kc, :], xt_r[:, kc, tqs])
                # q,k transposed: psum = wqkv_chunk.T @ xT
                for cp in range(4):
                    ps = ps_a.tile([128, TQ], f32, tag="ps_a")
                    for kc in range(NKC):
                        nc.tensor.matmul(
                            ps[:],
                            lhsT=wqkv_sb[:, kc, 128 * cp : 128 * (cp + 1)],
                            rhs=xts[:, kc, :],
                            start=(kc == 0),
                            stop=(kc == NKC - 1),
                        )
                    nc.vector.tensor_scalar_add(
                        qkT_sb[:, cp, tqs], ps[:], bqk_sb[:, cp : cp + 1]
                    )
                # v: psum = xT_chunk.T @ wv (+ bias via K=1 matmul)
                for tt in range(4 * tq, 4 * tq + 4):
                    psv = ps_a.tile([128, TQ], f32, tag="ps_a")
                    toff = 128 * tt - TQ * tq
                    for kc in range(NKC):
                        nc.tensor.matmul(
                            psv[:, 0:256],
                            lhsT=xts[:, kc, toff : toff + 128],
                            rhs=wqkv_sb[:, kc, 512:768],
                            start=(kc == 0),
                            stop=False,
                        )
                    nc.tensor.matmul(
                        psv[:, 0:256],
                        lhsT=ones_sb[0:1, 0:128],
                        rhs=bv_sb[0:1, :],
                        start=False,
                        stop=True,
                    )
                    for h in range(HPC):
                        col = 0 if h % 2 == 0 else 64
                        nc.vector.tensor_copy(
                            v_sb[:, tt, h, col : col + 64],
                            psv[:, 64 * h : 64 * h + 64],
                        )

                # ---- attention for this q-slice ---------------------------
                for pair in range(2):
                    hA, hB = 2 * pair, 2 * pair + 1
                    accA = ps_acc.tile([128, TQ], f32, tag="acc")
                    accB = ps_acc.tile([128, TQ], f32, tag="acc")
                    ntk = 4 * tq + 4
                    for tk in range(ntk):
                        d = tk - 4 * tq  # >= 0 on the diagonal block
                        q0 = 128 * d if d >= 0 else 0
                        w = TQ - q0
                        ks = slice(128 * tk, 128 * (tk + 1))
                        qs = slice(TQ * tq + q0, TQ * (tq + 1))
                        scA = ps_s.tile([128, TQ], f32, tag="sc")
                        scB = ps_s.tile([128, TQ], f32, tag="sc")
                        # two K=64 matmuls packed on row halves of the PE
                        nc.tensor.matmul(
                            scA[:, 0:w],
                            lhsT=qkT_sb[0:64, 2 + pair, ks],
                            rhs=qkT_sb[0:64, pair, qs],
                        )
                        nc.tensor.matmul(
                            scB[:, 0:w],
                            lhsT=qkT_sb[64:128, 2 + pair, ks],
                            rhs=qkT_sb[64:128, pair, qs],
                        )
                        ptA = pt_pool.tile([128, TQ], PV_DT, tag="pt")
                        ptB = pt_pool.tile([128, TQ], PV_DT, tag="pt")
                        nc.scalar.activation(ptA[:, 0:w], scA[:, 0:w], AF.Exp, scale=SCALE)
                        nc.scalar.activation(ptB[:, 0:w], scB[:, 0:w], AF.Exp, scale=SCALE)
                        if d >= 0:
                            # zero strictly-above-diagonal in the leading
                            # 128x128 block: keep where (j - p) >= 0
                            for pt_ in (ptA, ptB):
                                nc.gpsimd.affine_select(
                                    pt_[:, 0:128],
                                    pt_[:, 0:128],
                                    pattern=[[1, 128]],
                                    compare_op=ALU.is_ge,
                                    fill=0.0,
                                    base=0,
                                    channel_multiplier=-1,
                                )
                        if debug_taps and pair == 0 and tq == 0:
                            dtp = pt_pool.tile([128, TQ], f32, tag="dbgcp")
                            nc.vector.tensor_copy(dtp[:, 0:w], ptA[:, 0:w])
                            if q0 > 0:
                                nc.vector.memset(dtp[:, 0:q0], 0.0)
                            nc.sync.dma_start(dbg_pt[tk, :, :], dtp[:])
                        st = (tk == 0)
                        sp = (tk == ntk - 1)
                        # even head: out rows 0-63 att, row 64 denom
                        nc.tensor.matmul(
                            accA[0:65, q0:TQ],
                            lhsT=v_sb[:, tk, hA, 0:65],
                            rhs=ptA[:, 0:w],
                            start=st,
                            stop=sp,
                            skip_group_check=True,
                        )
                        # odd head: out row 0 denom, rows 64-127 att
                        nc.tensor.matmul(
                            accB[:, q0:TQ],
                            lhsT=v_sb[:, tk, hB, :],
                            rhs=ptB[:, 0:w],
                            start=st,
                            stop=sp,
                            skip_group_check=True,
                        )
                    # normalize head A (denominator at partition 64).
                    # reciprocal_approx_fast is broken at base_partition != 0
                    # on HW, so broadcast the raw denominator to rows 0-63
                    # first and take the reciprocal there.
                    denA = rec_pool.tile([128, TQ], f32r, tag="rec")
                    nc.vector.tensor_copy(denA[64:65, :], accA[64:65, :])
                    bcA = ps_a.tile([128, TQ], f32, tag="ps_a")
                    nc.tensor.matmul(
                        bcA[0:64, :],
                        lhsT=ones_sb[64:65, 0:64],
                        rhs=denA[64:65, :],
                    )
                    bcsA = bcs_pool.tile([128, TQ], f32, tag="bcs")
                    nc.scalar.activation(bcsA[0:64, :], bcA[0:64, :], AF.Copy)
                    nc.vector.reciprocal_approx_fast(
                        out=bcsA[0:64, :], in_=bcsA[0:64, :]
                    )
                    nc.vector.tensor_mul(
                        attT_sb[0:64, pair, tqs], accA[0:64, :], bcsA[0:64, :]
                    )
                    # normalize head B (denominator at partition 0)
                    recB = rec_pool.tile([128, TQ], f32, tag="rec")
                    nc.vector.reciprocal_approx_fast(
                        out=recB[0:1, :], in_=accB[0:1, :]
                    )
                    recBr = rec_pool.tile([128, TQ], f32r, tag="recr")
                    nc.vector.tensor_copy(recBr[0:1, :], recB[0:1, :])
                    bcB = ps_a.tile([128, TQ], f32, tag="ps_a")
                    nc.tensor.matmul(
                        bcB[:, :],
                        lhsT=onespad_sb[0:1, :],
                        rhs=recBr[0:1, :],
                    )
                    bcsB = bcs_pool.tile([128, TQ], f32, tag="bcs")
                    nc.scalar.activation(bcsB[64:128, :], bcB[64:128, :], AF.Copy)
                    nc.vector.tensor_mul(
                        attT_sb[64:128, pair, tqs], accB[64:128, :], bcsB[64:128, :]
                    )

                # ---- output projection for this t-slice -------------------
                for tt in range(4 * tq, 4 * tq + 4):
                    ts_ = slice(128 * tt, 128 * (tt + 1))
                    for nt in range(2):
                        ns = slice(512 * nt, 512 * (nt + 1))
                        pso = ps_a.tile([128, TQ], f32, tag="ps_a")
                        for hc in range(2):
                            nc.tensor.matmul(
                                pso[:],
                                lhsT=attT_sb[:, hc, ts_],
                                rhs=wproj_sb[:, hc, ns],
                                start=(hc == 0),
                                stop=False,
                            )
                        nc.tensor.matmul(
                            pso[:],
                            lhsT=ones_sb[0:1, 0:128],
                            rhs=bprojq_sb[0:1, ns],
                            start=False,
                            stop=True,
                        )
                        ot = ot_pool.tile([128, TQ], f32, tag="ot")
                        if nt == 0:
                            nc.vector.tensor_copy(ot[:], pso[:])
                        else:
                            nc.scalar.activation(ot[:], pso[:], AF.Copy)
                        nc.sync.dma_start(out_d[ts_, ns], ot[:])

            if debug_taps:
                nc.sync.dma_start(dbg_qkT[:, :, :], qkT_sb[:].bitcast(f32))
                nc.sync.dma_start(dbg_attT[:, :, :], attT_sb[:].bitcast(f32))

    nc.compile()
    return nc


def _shard_inputs(x, w_qkv, b_qkv, w_proj, b_proj):
    """Full inputs -> per-core input maps. Core c = (batch b=c//4, group g=c%4)."""
    in_maps = []
    xts = [np.ascontiguousarray(x[b].T) for b in range(B)]
    bprojq = np.ascontiguousarray((0.25 * b_proj).reshape(1, C).astype(np.float32))
    for core in range(NCORES):
        b, g = divmod(core, 4)
        qs = slice(256 * g, 256 * (g + 1))
        ks = slice(C + 256 * g, C + 256 * (g + 1))
        vs = slice(2 * C + 256 * g, 2 * C + 256 * (g + 1))
        wqkv = np.ascontiguousarray(
            np.concatenate([w_qkv[:, qs], w_qkv[:, ks], w_qkv[:, vs]], axis=1)
        )
        bqk = np.ascontiguousarray(
            np.stack(
                [
                    b_qkv[qs][0:128],
                    b_qkv[qs][128:256],
                    b_qkv[ks][0:128],
                    b_qkv[ks][128:256],
                ],
                axis=1,
            )
        )
        bv = np.ascontiguousarray(b_qkv[vs].reshape(1, 256))
        wproj = np.ascontiguousarray(w_proj[256 * g : 256 * (g + 1), :])
        in_maps.append(
            {
                "xt": xts[b],
                "wqkv": wqkv.astype(np.float32),
                "bqk": bqk.astype(np.float32),
                "bv": bv.astype(np.float32),
                "wproj": wproj.astype(np.float32),
                "bprojq": bprojq,
            }
        )
    return in_maps


def kernel(x, w_qkv, b_qkv, w_proj, b_proj):
    x = np.asarray(x, dtype=np.float32)
    w_qkv = np.asarray(w_qkv, dtype=np.float32)
    b_qkv = np.asarray(b_qkv, dtype=np.float32)
    w_proj = np.asarray(w_proj, dtype=np.float32)
    b_proj = np.asarray(b_proj, dtype=np.float32)

    if "nc" not in _CACHE:
        _CACHE["nc"] = build_nc()
    nc = _CACHE["nc"]

    in_maps = _shard_inputs(x, w_qkv, b_qkv, w_proj, b_proj)
    res = run_bass_kernel_spmd(nc, in_maps, list(range(NCORES)))
    out = np.empty((B, T, C), dtype=np.float32)
    for b in range(B):
        acc = res.results[4 * b]["out"].astype(np.float32)
        for g in range(1, 4):
            acc = acc + res.results[4 * b + g]["out"]
        out[b] = acc
    return out
